# revision 14
# baseline (speedup 1.0000x reference)
"""Bass/Trainium2 kernel for nn_BayesianSTDPAdaptive (8-core SPMD).

Math (reference semantics, restructured for the engines):
  per step k (200 steps of TIME_BATCH=10):
    corr_k[o,i] = sum_b spikes[k,b,o] * psp[k,b,i]     (PE matmul, K=10)
    e      = exp(-w)                                    (ACT Exp)
    u      = corr_k * e                                 (DVE TT)
    dw     = clip(u - tot_k[o], -10, 10)                (DVE TS fused)
    w     += mu * dw                                    (DVE STT + TT)
    d      = w - w1;  w1 += MOM*d                       (DVE TT + STT)
    vh     = (1-MOM)*vh + MOM*(1-MOM)/2 * d^2           (ACT Square + DVE STT)
             [vh tracks (w2 - w1^2)/2 exactly, by algebra]
    mu     = clip((tanh(w1/2)+1) * vh, 1e-6, UB)        (ACT Tanh + DVE STT+TS)
             [since sigmoid(x) = (tanh(x/2)+1)/2 and mu = (w2-w1^2)*sigmoid(w1)]
  All ACT functions (Exp/Square/Tanh/Copy) live in the one `exp_and_others`
  table set, so there are no table reloads inside the scan.

  The bias path has the same per-element recurrence, so it rides along as an
  extra column: psp column 125 is 1.0, making corr[:,125] = tot_k, and
  u[:,125] gets the (tot - totsum) correction before the shared subtract.

Sharding: input dim I=1000 split 125/core across 8 cores; spikes and the
bias column are replicated (bias dynamics are independent of weight columns,
so every core computes identical bias state; core 0's copy is returned).

Inputs are packed into two DRAM tensors ("seq" and "stat") so the kernel
issues exactly two input DMAs — walrus allows only one sync-wait on some
instruction formats, and fewer producers keeps every wait list short.
"""

from contextlib import ExitStack

import numpy as np

import concourse.bass as bass
import concourse.mybir as mybir
import concourse.tile as tile
from concourse.bass import _add_dep_helper
from concourse.bass_utils import run_bass_kernel_spmd

O, I, T, TB = 100, 1000, 2000, 10
ITERS = T // TB          # 200
NCORES = 8
S = I // NCORES          # 125 weight columns per core
F = S + 1                # +1 fused bias column
MOM = 0.001
MAXD = 10.0
BASE_MU_W = 1.0
BASE_MU_B = 0.5
MIN_MU = 1e-6

f32 = mybir.dt.float32
Alu = mybir.AluOpType
Act = mybir.ActivationFunctionType


def _build(iters: int) -> bass.Bass:
    nc = bass.Bass()

    # seq: [psp_ext | spikes] along the free dim, partition dim = TIME_BATCH
    seq_d = nc.declare_dram_parameter(
        "seq", [TB, iters * (F + O)], f32, isOutput=False
    )
    # stat: [w0 | w0 | vh0 | mu0 | totsum_bcast] along the free dim
    stat_d = nc.declare_dram_parameter(
        "stat", [O, 4 * F + iters], f32, isOutput=False
    )
    wout_d = nc.declare_dram_parameter("wout", [O, F], f32, isOutput=True)

    s2h = float(np.sqrt(MOM * (1.0 - MOM) / 2.0))

    with tile.TileContext(nc) as tc, ExitStack() as ctx:
        inp = ctx.enter_context(tc.tile_pool(name="inp", bufs=1))
        state = ctx.enter_context(tc.tile_pool(name="state", bufs=1))
        tmp = ctx.enter_context(tc.tile_pool(name="tmp", bufs=3))
        # 1x1 wait-carrier tiles; deep pool so carrier slot-reuse deps stay
        # covered by the real ACT ops' own-engine waits.
        car = ctx.enter_context(tc.tile_pool(name="car", bufs=8))
        psum = ctx.enter_context(tc.tile_pool(name="psum", bufs=6, space="PSUM"))

        # ---- load inputs (2 DMAs) ----
        seq_sb = inp.tile([TB, iters * (F + O)], f32)
        dma_in1 = nc.sync.dma_start(seq_sb[:], seq_d[:])
        stat_sb = inp.tile([O, 4 * F + iters], f32)
        dma_in2 = nc.sync.dma_start(stat_sb[:], stat_d[:])

        psp = seq_sb[:, : iters * F]
        spk = seq_sb[:, iters * F :]
        tsb = stat_sb[:, 4 * F : 4 * F + iters]

        # separate state tiles (slices of one tile would false-serialize)
        w = state.tile([O, F], f32)
        nc.vector.tensor_copy(w[:], stat_sb[:, 0:F])
        w1 = state.tile([O, F], f32)
        nc.vector.tensor_copy(w1[:], stat_sb[:, F : 2 * F])
        vh = state.tile([O, F], f32)
        nc.vector.tensor_copy(vh[:], stat_sb[:, 2 * F : 3 * F])
        mu = state.tile([O, F], f32)
        nc.vector.tensor_copy(mu[:], stat_sb[:, 3 * F : 4 * F])

        # Keep the ACT stream in emission order: pool-slot release deps on
        # ACT ops then stay subsumed by earlier cross-engine waits, keeping
        # every instruction at <=1 sync-wait (walrus format limit).
        prev_act = [None]

        def act_chain(inst):
            if prev_act[0] is not None:
                _add_dep_helper(
                    inst.ins, prev_act[0].ins, sync=False, reason="pin ACT order"
                )
            prev_act[0] = inst
            return inst


        # ---- the scan ----
        last_w_update = None
        for k in range(iters):
            cps = psum.tile([O, F], f32, tag="corr_ps")
            last_mm = nc.tensor.matmul(
                cps[:],
                spk[:, k * O : (k + 1) * O],
                psp[:, k * F : (k + 1) * F],
                start=True,
                stop=True,
            )
            # PE-tick carrier: a real 1x1 ACT copy of the matmul output, so
            # the full-tile copy below needs only its own-engine slot wait
            # (walrus allows a single sync-wait per instruction).
            pec = car.tile([1, 1], f32, tag="pec")
            act_chain(nc.scalar.copy(pec[:], cps[0:1, 0:1]))
            corr = tmp.tile([O, F], f32, tag="corr")
            act_chain(nc.scalar.copy(corr[:], cps[:]))

            e = tmp.tile([O, F], f32, tag="e")
            act_chain(nc.scalar.activation(e[:], w[:], Act.Exp, bias=0.0, scale=-1.0))

            u = tmp.tile([O, F], f32, tag="u")
            nc.vector.tensor_tensor(u[:], corr[:], e[:], Alu.mult)
            # bias column: u125 <- (u125 - totsum_k) + tot_k
            nc.vector.scalar_tensor_tensor(
                u[:, S : S + 1],
                u[:, S : S + 1],
                tsb[:, k : k + 1],
                corr[:, S : S + 1],
                Alu.subtract,
                Alu.add,
            )
            # t = (u - tot_k) min 10   (in place)
            nc.vector.tensor_scalar(
                u[:], u[:], corr[:, S : S + 1], MAXD, Alu.subtract, Alu.min
            )
            # g = (t max -10) * mu     (in place)
            nc.vector.scalar_tensor_tensor(
                u[:], u[:], -MAXD, mu[:], Alu.max, Alu.mult
            )
            last_w_update = nc.vector.tensor_tensor(w[:], w[:], u[:], Alu.add)

            d = tmp.tile([O, F], f32, tag="d")
            d_inst = nc.vector.tensor_tensor(d[:], w[:], w1[:], Alu.subtract)
            w1_inst = nc.vector.scalar_tensor_tensor(
                w1[:], d[:], MOM, w1[:], Alu.mult, Alu.add
            )
            # w1-tick carrier: covers the DVE ticks of both the d and w1
            # updates for the p2/th activations below.
            w1c = car.tile([1, 1], f32, tag="w1c")
            act_chain(nc.scalar.copy(w1c[:], w1[0:1, 0:1]))
            p2 = tmp.tile([O, F], f32, tag="p2")
            act_chain(nc.scalar.activation(p2[:], d[:], Act.Square, bias=0.0, scale=s2h))
            nc.vector.scalar_tensor_tensor(
                vh[:], vh[:], 1.0 - MOM, p2[:], Alu.mult, Alu.add
            )
            th = tmp.tile([O, F], f32, tag="th")
            act_chain(nc.scalar.activation(th[:], w1[:], Act.Tanh, bias=0.0, scale=0.5))
            # split (th+1)*vh into two ops: walrus allows only one sync-wait
            # on the TensorScalarPtr format, and the fused form needs both an
            # ACT wait (th) and a DVE wait (vh written immediately before).
            th1 = tmp.tile([O, F], f32, tag="th1")
            nc.vector.tensor_scalar(th1[:], th[:], 1.0, None, Alu.add)
            nc.vector.tensor_tensor(mu[:], th1[:], vh[:], Alu.mult)
            nc.vector.tensor_scalar(
                mu[:], mu[:], MIN_MU, BASE_MU_W, Alu.max, Alu.min
            )
            last_dve = nc.vector.tensor_single_scalar(
                mu[:, S : S + 1], mu[:, S : S + 1], BASE_MU_B, Alu.min
            )

        dma_out = nc.sync.dma_start(wout_d[:], w[:])

        # The auto-emitted kernel-tail drain waits on every engine + DMA
        # queue, exceeding the ISA's per-instruction sync-wait slots.
        # Pre-cover SP's observed clock with a chain of single-wait nops so
        # the drain's waits are all elided.
        for leaf in (dma_in1, dma_in2, last_mm, prev_act[0], last_dve, dma_out):
            nop = nc.sync.nop()
            _add_dep_helper(nop.ins, leaf.ins, sync=True, reason="tail fold")

    return nc


def _prep_inputs(input_psp, output_spikes, weights, biases, iters):
    """Shard + lay out host arrays for the 8 cores."""
    psp3 = (
        np.ascontiguousarray(input_psp[: iters * TB])
        .reshape(iters, TB, I)
        .transpose(1, 0, 2)
    )  # (TB, iters, I)
    spk3 = (
        np.ascontiguousarray(output_spikes[: iters * TB])
        .reshape(iters, TB, O)
        .transpose(1, 0, 2)
    )  # (TB, iters, O)
    tsum = output_spikes[: iters * TB].reshape(iters, TB * O).sum(axis=1)

    stat = np.empty((O, 4 * F + iters), np.float32)
    stat[:, 2 * F : 3 * F] = 0.5          # vh0
    stat[:, 2 * F + S] = 0.25
    stat[:, 3 * F : 4 * F] = BASE_MU_W    # mu0
    stat[:, 3 * F + S] = BASE_MU_B
    stat[:, 4 * F :] = tsum[None, :].astype(np.float32)

    in_maps = []
    for c in range(NCORES):
        seq = np.empty((TB, iters, F + O), np.float32)
        seq[:, :, :S] = psp3[:, :, c * S : (c + 1) * S]
        seq[:, :, S] = 1.0
        # spikes go after all psp blocks in the free dim
        statc = stat.copy()
        statc[:, :S] = weights[:, c * S : (c + 1) * S]
        statc[:, S] = biases
        statc[:, F : F + S] = weights[:, c * S : (c + 1) * S]
        statc[:, F + S] = biases
        seq_flat = np.empty((TB, iters * (F + O)), np.float32)
        seq_flat[:, : iters * F] = seq[:, :, :F].reshape(TB, iters * F)
        seq_flat[:, iters * F :] = np.ascontiguousarray(spk3).reshape(TB, iters * O)
        in_maps.append({"seq": seq_flat, "stat": statc})
    return in_maps


def run_device(input_psp, output_spikes, weights, biases, iters=ITERS, **spmd_kwargs):
    """Build, run on the 8 NeuronCores, gather. Returns ((w, b), results)."""
    input_psp = np.asarray(input_psp, np.float32)
    output_spikes = np.asarray(output_spikes, np.float32)
    weights = np.asarray(weights, np.float32)
    biases = np.asarray(biases, np.float32)
    in_maps = _prep_inputs(input_psp, output_spikes, weights, biases, iters)
    nc = _build(iters)
    res = run_bass_kernel_spmd(nc, in_maps, list(range(NCORES)), **spmd_kwargs)
    wouts = [np.asarray(res.results[c]["wout"]) for c in range(NCORES)]
    w_full = np.concatenate([wo[:, :S] for wo in wouts], axis=1)
    b_full = np.ascontiguousarray(wouts[0][:, S])
    return (w_full, b_full), res


def kernel(input_psp, output_spikes, weights, biases):
    (w_full, b_full), _ = run_device(input_psp, output_spikes, weights, biases)
    return w_full, b_full
